# revision 35
# baseline (speedup 1.0000x reference)
"""Trainium2 Bass kernel for a 2-layer LSTM (B=256, T=512, IN=8, H=512) + FC head.

Strategy: data-parallel over batch (32 per core x 8 cores). Per core, one
software-pipelined loop over supersteps s: layer-0 computes step s while
layer-1 computes step s-1 (so layer-1's matmuls never wait on this step's
elementwise chain). Gates are computed with the batch (32) as the PE
stationary free dim, one gate per PE column-group, so the four gates land
partition-stacked [i|f|o|g] x 32batch in PSUM and biases/input-projections
are folded in as extra accumulating matmuls (ones-row trick). The scalar
engine applies sigmoid/tanh straight out of PSUM, the vector engine does the
c/h updates, and h is transposed back to [hidden, batch] stationary layout
with PE identity matmuls.

The superstep loop is a hardware For_i loop (SPB supersteps per body) so the
program size is O(1) in T instead of O(T): the fully-unrolled version's NEFF
was ~30k instructions and its load/transfer time dominated wall clock.
"""

import sys
from contextlib import ExitStack

import numpy as np

try:
    import concourse.bass as bass  # noqa: F401
except ImportError:
    sys.path.insert(0, "/opt/trn_rl_repo")

import ml_dtypes
import concourse.bacc as bacc
import concourse.bass as bass
import concourse.mybir as mybir
import concourse.tile as tile
from concourse.bass import ds
from concourse.bass_utils import run_bass_kernel_spmd
from concourse.masks import make_identity
from concourse.tile_rust import add_dep_helper

B, T, IN, H = 256, 512, 8, 512
N_CORES = 8
BC = B // N_CORES  # 32 batch rows per core
MAXV, MINV = 4.2, 2.5

AF = mybir.ActivationFunctionType
F32 = mybir.dt.float32

# Stream dtype for matmul operands (weights, h, u). bf16 = 1 PE cycle/row.
DT = mybir.dt.bfloat16
NPDT = ml_dtypes.bfloat16

# Gate blocks in torch order: i[0:H], f[H:2H], g[2H:3H], o[3H:4H].
# On-chip layout order is [i, f, o, g] so the three sigmoids are one
# contiguous 96-partition block. PERM maps layout order -> torch rows.
PERM = np.concatenate(
    [np.arange(0, H), np.arange(H, 2 * H), np.arange(3 * H, 4 * H),
     np.arange(2 * H, 3 * H)]
)


def _shuffle_kxn(w_t: np.ndarray, n: int) -> np.ndarray:
    """[512, n] (contraction-major) -> [128, 4, n] SBUF layout (chunk k = rows
    128k:128k+128 on partition p)."""
    return np.ascontiguousarray(w_t.reshape(4, 128, n).transpose(1, 0, 2))


def build_program(t_steps: int, reps: int = 1, virtual_steps: int = 0):
    """Emit the per-core Bass program. All 8 cores run this same program.
    Bacc (not plain Bass): its finalize() runs the compile passes that move
    matmul waits onto ldweights and split multi-wait instructions into
    event semaphores -- hardware instructions encode only one sync wait.

    virtual_steps > t_steps builds a TIMING variant: the identical program
    except the hardware loop runs (virtual_steps - spb)/spb iterations with
    the u_t staging copy always reading block 0 (data values recycle; the
    per-iteration instruction stream and timing are unchanged). Output is
    then numerically meaningless -- timing only."""
    nc = bacc.Bacc()

    # Supersteps per hardware-loop body. The loop covers s in [SPB, t_steps);
    # supersteps 0..SPB-1 are the peeled prologue (s=0 has no layer-1 work)
    # and s=t_steps the peeled epilogue (layer-1 only).
    spb = 8 if (t_steps % 8 == 0 and t_steps >= 16) else 1

    # --- DRAM parameters (declaration order = in_map key order is by name) ---
    dp = nc.declare_dram_parameter
    wrec0_d = dp("wrec0", [128, 4, 4 * H], DT, isOutput=False)
    wx1_d = dp("wx1", [128, 4, 4 * H], DT, isOutput=False)
    wrec1_d = dp("wrec1", [128, 4, 4 * H], DT, isOutput=False)
    w0aug_d = dp("w0aug", [IN + 1, 4 * H], DT, isOutput=False)
    bias1_d = dp("bias1", [1, 4 * H], DT, isOutput=False)
    ut_d = dp("ut", [IN + 1, t_steps + 1, BC], DT, isOutput=False)
    fcw0_d = dp("fcw0", [128, 4, 256], DT, isOutput=False)
    fcb0_d = dp("fcb0", [1, 256], DT, isOutput=False)
    fcw1_d = dp("fcw1", [128, 2, 2], DT, isOutput=False)
    fcb1_d = dp("fcb1", [1, 2], DT, isOutput=False)
    out_d = dp("out", [BC, 2], F32, isOutput=True)

    with tile.TileContext(nc) as tc, ExitStack() as ctx:
        const = ctx.enter_context(tc.tile_pool(name="const", bufs=1))
        st = ctx.enter_context(tc.tile_pool(name="state", bufs=8))
        work = ctx.enter_context(tc.tile_pool(name="work", bufs=4))
        pp = ctx.enter_context(tc.tile_pool(name="ps", bufs=3, space="PSUM"))
        pt = ctx.enter_context(tc.tile_pool(name="pst", bufs=1, space="PSUM"))
        pf = ctx.enter_context(tc.tile_pool(name="psf", bufs=1, space="PSUM"))
        up = ctx.enter_context(tc.tile_pool(name="ustage", bufs=1))

        # --- resident weights / inputs ---
        wrec0 = const.tile([128, 4, 4 * H], DT, tag="wrec0")
        wx1 = const.tile([128, 4, 4 * H], DT, tag="wx1")
        wrec1 = const.tile([128, 4, 4 * H], DT, tag="wrec1")
        w0aug = const.tile([IN + 1, 4 * H], DT, tag="w0aug")
        bias1 = const.tile([1, 4 * H], DT, tag="bias1")
        ut = const.tile([IN + 1, t_steps + 1, BC], DT, tag="ut")
        fcw0 = const.tile([128, 4, 256], DT, tag="fcw0")
        fcb0 = const.tile([1, 256], DT, tag="fcb0")
        fcw1 = const.tile([128, 2, 2], DT, tag="fcw1")
        fcb1 = const.tile([1, 2], DT, tag="fcb1")
        for sb, d in ((wrec0, wrec0_d), (wx1, wx1_d), (wrec1, wrec1_d),
                      (w0aug, w0aug_d), (bias1, bias1_d), (ut, ut_d),
                      (fcw0, fcw0_d), (fcb0, fcb0_d), (fcw1, fcw1_d),
                      (fcb1, fcb1_d)):
            nc.sync.dma_start(sb[:], d[:])

        # Preheat: 1-element matmuls touching every DMA-loaded tensor, so PE
        # observes each DMA-HW queue semaphore up front. Otherwise the first
        # PE consumer of a tensor carries an extra DMA wait on top of its
        # ACT/PE waits, and a Matmult encodes at most two sync waits.
        pre = pf.tile([1, 1], F32, tag="pfence")
        preheats = []
        for ap in (wrec0[0:1, 0, 0:1], wx1[0:1, 0, 0:1], wrec1[0:1, 0, 0:1],
                   w0aug[0:1, 0:1], bias1[0:1, 0:1], ut[0:1, 0, 0:1],
                   fcw0[0:1, 0, 0:1], fcb0[0:1, 0:1], fcw1[0:1, 0, 0:1],
                   fcb1[0:1, 0:1]):
            preheats.append(
                nc.tensor.matmul(pre[:], ap, ap, start=True, stop=True))

        ident = const.tile([32, 32], DT, tag="ident")
        make_identity(nc, ident[:])
        ones = const.tile([1, BC], DT, tag="ones")
        nc.vector.memset(ones[:], 1.0)
        out_bias = const.tile([BC, 1], F32, tag="out_bias")
        nc.vector.memset(out_bias[:], (MAXV - MINV) / 2 + MINV)

        early_mms = []

        def gps(ps, g):
            return ps[32 * g:32 * g + 32, :]

        # Mutable pipeline state shared by superstep emissions.
        sv = {}

        def start_l0(ut_ap, collect=None):
            """Open next step's layer-0 gate accumulation: [u_t, 1] @ w0aug
            (biases ride the ones row). Independent of any recent h, so the
            PE chews it while ACT/DVE run the current elementwise chains.
            The sync dep on the previous superstep's sigmoid throttles the
            scheduler: these matmuls have no data deps, and unthrottled it
            hoists several supersteps' opens, blowing the PSUM ring."""
            ps0 = pp.tile([128, H], F32, tag="ps0", name="ps0")
            for g in range(4):
                r = nc.tensor.matmul(gps(ps0, g), ut_ap,
                                     w0aug[:, 512 * g:512 * g + 512],
                                     start=True, stop=False,
                                     tile_position=(0, 32 * g))
                if g == 0 and sv.get("sig0_prev") is not None:
                    add_dep_helper(r.ins, sv["sig0_prev"].ins, sync=True,
                                   reason="throttle-ps0-open")
                if collect is not None:
                    collect.append(r)
            sv["ps0n"] = ps0

        def start_l1(collect=None):
            """Open next l1 step's gate accumulation: bias1 + h0 @ wx1.
            h0 here is one superstep old -- ready long before this runs."""
            ps1 = pp.tile([128, H], F32, tag="ps1", name="ps1")
            for g in range(4):
                r = nc.tensor.matmul(gps(ps1, g), ones[:],
                                     bias1[:, 512 * g:512 * g + 512],
                                     start=True, stop=False,
                                     tile_position=(0, 32 * g))
                if g == 0 and sv.get("sig1_prev") is not None:
                    add_dep_helper(r.ins, sv["sig1_prev"].ins, sync=True,
                                   reason="throttle-ps1-open")
                if collect is not None:
                    collect.append(r)
            for k in range(4):
                for g in range(4):
                    r = nc.tensor.matmul(gps(ps1, g),
                                         sv["h0T"][:, 32 * k:32 * k + 32],
                                         wx1[:, k, 512 * g:512 * g + 512],
                                         start=False, stop=False,
                                         tile_position=(0, 32 * g))
                    if collect is not None:
                        collect.append(r)
            sv["ps1n"] = ps1

        def emit_superstep(ut_next_ap, l0, l1, l0n, l1n, collect=None):
            """One steady-state superstep s, layers phase-shifted so each
            layer's elementwise chain overlaps the other layer's matmuls:

              PE : wrec0(s) | wrec1(s-2) | start l0(s+1), l1(s-1) | transposes
              ACT:      tanh/sig l0 | tanh/sig l1 | tanh-c l0 | tanh-c l1
              DVE:          c/h updates l0 | c/h updates l1 | h*T copies

            Matmuls go round-robin across the four gates: each gate owns one
            32-wide PE column-group (tile_position) and col-groups stream
            CONCURRENTLY through separate XBUSes when adjacent in program
            order (measured ~2.4-3x for 3-4 way col tiling)."""

            def gmm(*args, **kw):
                r = nc.tensor.matmul(*args, **kw)
                if collect is not None:
                    collect.append(r)
                return r

            ps0, ps1 = sv.get("ps0n"), sv.get("ps1n")
            # --- PE: close layer-0 gates for step s: + h0(s-1) @ wrec0 ---
            if l0:
                for k in range(4):
                    for g in range(4):
                        gmm(gps(ps0, g), sv["h0T"][:, 32 * k:32 * k + 32],
                            wrec0[:, k, 512 * g:512 * g + 512],
                            start=False, stop=(k == 3),
                            tile_position=(0, 32 * g))
            # --- ACT: layer-0 nonlinearities straight out of PSUM. Sigmoid
            # first: f*c_prev needs only the sigmoid, so it runs on DVE
            # while ACT still computes tanh(g). ---
            gt = work.tile([BC, 2 * H], DT, tag="gt")
            sig = work.tile([96, 2 * H], DT, tag="sig")  # [i|f|o] x batch
            if l0:
                nc.scalar.activation(sig[:, 0:H], ps0[0:96, :], AF.Sigmoid)
                sv["sig0_prev"] = nc.scalar.activation(
                    gt[:, 0:H], ps0[96:128, :], AF.Tanh)
            # --- PE: close layer-1 gates for step s-2: + h1(s-3) @ wrec1 ---
            if l1:
                for k in range(4):
                    for g in range(4):
                        gmm(gps(ps1, g), sv["h1T"][:, 32 * k:32 * k + 32],
                            wrec1[:, k, 512 * g:512 * g + 512],
                            start=False, stop=(k == 3),
                            tile_position=(0, 32 * g))
            # --- DVE: layer-0 c update ---
            ig = work.tile([BC, 2 * H], DT, tag="ig")
            fcp = work.tile([BC, 2 * H], F32, tag="fcp")
            c_new = st.tile([64, 2 * H], F32, tag="c", name="c")
            if l0:
                nc.vector.tensor_mul(fcp[:, 0:H], sig[32:64, 0:H],
                                     sv["c_prev"][32:64, 0:H])
                nc.vector.tensor_mul(ig[:, 0:H], sig[0:32, 0:H], gt[:, 0:H])
                nc.vector.tensor_add(c_new[32:64, 0:H], fcp[:, 0:H],
                                     ig[:, 0:H])
            # --- ACT: layer-1 nonlinearities ---
            if l1:
                nc.scalar.activation(sig[:, H:2 * H], ps1[0:96, :], AF.Sigmoid)
                sv["sig1_prev"] = nc.scalar.activation(
                    gt[:, H:2 * H], ps1[96:128, :], AF.Tanh)
            elif l0:
                # Layer-1 idle (pipeline fill): carry its c half forward as
                # zero so the first l1 superstep reads c=0.
                nc.vector.memset(c_new[32:64, H:2 * H], 0.0)
            # --- PE: open next supersteps' gate accumulations ---
            if l0n:
                start_l0(ut_next_ap, collect)
            if l1n:
                start_l1(collect)
            # --- DVE: layer-1 c update ---
            if l1:
                nc.vector.tensor_mul(fcp[:, H:2 * H], sig[32:64, H:2 * H],
                                     sv["c_prev"][32:64, H:2 * H])
                nc.vector.tensor_mul(ig[:, H:2 * H], sig[0:32, H:2 * H],
                                     gt[:, H:2 * H])
            # --- ACT tanh(c) + DVE h for layer 0, then layer 1 ---
            th = work.tile([96, 2 * H], DT, tag="th")
            hn = work.tile([BC, 2 * H], DT, tag="hn")
            hps = pt.tile([128, 256], DT, tag="hps")
            if l0:
                nc.scalar.activation(th[64:96, 0:H], c_new[32:64, 0:H],
                                     AF.Tanh)
            if l1:
                nc.vector.tensor_add(c_new[32:64, H:2 * H], fcp[:, H:2 * H],
                                     ig[:, H:2 * H])
            if l0:
                nc.vector.tensor_mul(hn[:, 0:H], sig[64:96, 0:H],
                                     th[64:96, 0:H])
                # PE transpose back to [hid, batch] stationary layout; DVE
                # (not ACT: 0.11us vs 0.4us) copies PSUM -> SBUF.
                for k in range(4):
                    nc.tensor.transpose(hps[:, 32 * k:32 * k + 32],
                                        hn[:, 128 * k:128 * k + 128], ident[:])
                sv["h0T"] = st.tile([128, 128], DT, tag="h0T", name="h0T")
                nc.vector.tensor_copy(sv["h0T"][:], hps[:, 0:128])
            if l1:
                nc.scalar.activation(th[64:96, H:2 * H], c_new[32:64, H:2 * H],
                                     AF.Tanh)
                nc.vector.tensor_mul(hn[:, H:2 * H], sig[64:96, H:2 * H],
                                     th[64:96, H:2 * H])
                for k in range(4):
                    nc.tensor.transpose(hps[:, 128 + 32 * k:160 + 32 * k],
                                        hn[:, H + 128 * k:H + 128 * k + 128],
                                        ident[:])
                sv["h1T"] = st.tile([128, 128], DT, tag="h1T", name="h1T")
                nc.vector.tensor_copy(sv["h1T"][:], hps[:, 128:256])
            sv["c_prev"] = c_new

            # PE-clock fence: a 1x1 matmul (own PSUM bank) + DVE copy makes
            # DVE observe a PE tick that transitively follows this step's
            # transposes, so later WAR-vs-PE waits on reused work slots are
            # pre-covered and TensorTensors keep to their one sync wait.
            lo = 0 if l0 else H
            pfence = pf.tile([1, 1], F32, tag="pfence")
            nc.tensor.matmul(pfence[:], hn[0:1, lo:lo + 1],
                             ones[0:1, 0:1], start=True, stop=True)
            fence = work.tile([1, 1], F32, tag="fence")
            nc.vector.tensor_copy(fence[:], pfence[:])

        for rep in range(reps):
          # --- initial state (h/c zero) ---
          # c lives on partitions 32:64 so that f*c pairs sigmoid(f) (block
          # 32:64 of sig_t) with an equal base partition -- walrus requires
          # SBUF x SBUF tensor_tensor inputs to share their start partition.
          sv["h0T"] = st.tile([128, 128], DT, tag="h0T", name="h0T")
          sv["h1T"] = st.tile([128, 128], DT, tag="h1T", name="h1T")
          sv["c_prev"] = st.tile([64, 2 * H], F32, tag="c", name="c")
          nc.vector.memset(sv["h0T"][:], 0.0)
          nc.vector.memset(sv["h1T"][:], 0.0)
          nc.vector.memset(sv["c_prev"][32:64, :], 0.0)

          # Superstep s runs: close l0 gates for step s, close l1 gates for
          # step s-2, and open the next supersteps' gate accumulations
          # (l0 step s+1, l1 step s-1 -- whose wx1 consumes the h0 produced
          # one superstep ago, so it never waits on a fresh h).
          # Prime l0 step 0, then prologue supersteps 0..spb-1 (unrolled).
          start_l0(ut[:, 0, :], collect=early_mms)
          for s in range(spb):
              emit_superstep(ut[:, s + 1, :], True, s >= 2, True, s >= 1,
                             collect=early_mms if s <= 1 else None)

          # Steady state: hardware loop over supersteps [spb, t_steps).
          # walrus does not support register offsets in ldweights, so the
          # body's u_t stationaries are staged: one DVE copy per body pulls
          # ut[:, iv:iv+spb+1, :] into a fixed tile the matmuls slice
          # statically (spb+1 wide: superstep iv+k opens step iv+k+1; ut is
          # padded by one zero step so the final open reads in-bounds).
          # hint_engines=(PE,): the body is >256 PE instructions (more than
          # one IRAM block), so arm the back-edge prefetcher.
          loop_end = max(virtual_steps, t_steps)
          if loop_end > spb:
              with tc.For_i(spb, loop_end, spb,
                            hint_engines=(mybir.EngineType.PE,)) as iv:
                  ust = up.tile([IN + 1, (spb + 1) * BC], DT, tag="ust",
                                name="ust")
                  if virtual_steps > t_steps:
                      nc.vector.tensor_copy(ust[:], ut[:, 0:spb + 1, :])
                  else:
                      nc.vector.tensor_copy(ust[:], ut[:, ds(iv, spb + 1), :])
                  for k in range(spb):
                      emit_superstep(ust[:, BC * (k + 1):BC * (k + 2)],
                                     True, True, True, True)

          # Epilogue supersteps t_steps, t_steps+1: layer-1 only (drains
          # the lag-2 pipeline: l1 steps t_steps-2 and t_steps-1).
          emit_superstep(None, False, True, False, True)
          emit_superstep(None, False, True, False, False)

        # Preheat matmuls must precede the early-step matmuls in PE program
        # order so the DMA-queue semaphore waits land on the preheats.
        for p in preheats:
            for m_ in early_mms:
                add_dep_helper(p.ins, m_.ins, sync=True, reason="preheat-first")

        # --- FC head on final h1 (h1T holds it in [hid, batch] layout) ---
        psf = pp.tile([128, H], F32, tag="ps0", name="psf")
        nc.tensor.matmul(psf[0:BC, 0:256], ones[:], fcb0[:],
                         start=True, stop=False)
        for k in range(4):
            nc.tensor.matmul(psf[0:BC, 0:256], sv["h1T"][:, 32 * k:32 * k + 32],
                             fcw0[:, k, :], start=False, stop=(k == 3))
        z = work.tile([BC, 256], DT, tag="z")
        nc.scalar.activation(z[:], psf[0:BC, 0:256], AF.Tanh)
        zps = pt.tile([128, 256], DT, tag="hps", name="zps")
        for k in range(2):
            nc.tensor.transpose(zps[:, 32 * k:32 * k + 32],
                                z[:, 128 * k:128 * k + 128], ident[:])
        zt = work.tile([128, 64], DT, tag="zt")
        nc.scalar.copy(zt[:], zps[:, 0:64])
        psg = pp.tile([128, H], F32, tag="ps1", name="psg")
        nc.tensor.matmul(psg[0:BC, 0:2], ones[:], fcb1[:],
                         start=True, stop=False)
        for k in range(2):
            nc.tensor.matmul(psg[0:BC, 0:2], zt[:, 32 * k:32 * k + 32],
                             fcw1[:, k, :], start=False, stop=(k == 1))
        res = work.tile([BC, 2], F32, tag="res")
        # out = (x + 1)/2 * (MAXV-MINV) + MINV = 0.85*x + 3.35
        nc.scalar.activation(res[:], psg[0:BC, 0:2], AF.Identity,
                             bias=out_bias[:], scale=(MAXV - MINV) / 2)
        nc.sync.dma_start(out_d[:], res[:])

    nc.finalize()
    return nc


def prep_inputs(inputs: dict, t_steps: int):
    """Host-side: transpose/permute/convert everything into device layouts.
    Returns the per-core in_maps list."""
    f = {k: np.asarray(v, np.float32) for k, v in inputs.items()}
    wrec0 = _shuffle_kxn(f["w_hh_0"].T[:, PERM], 4 * H).astype(NPDT)
    wx1 = _shuffle_kxn(f["w_ih_1"].T[:, PERM], 4 * H).astype(NPDT)
    wrec1 = _shuffle_kxn(f["w_hh_1"].T[:, PERM], 4 * H).astype(NPDT)
    w0aug = np.concatenate(
        [f["w_ih_0"].T, (f["b_ih_0"] + f["b_hh_0"])[None, :]], axis=0
    )[:, PERM].astype(NPDT)
    bias1 = (f["b_ih_1"] + f["b_hh_1"])[PERM][None, :].astype(NPDT)
    fcw0 = _shuffle_kxn(f["fc_w0"].T, 256).astype(NPDT)
    fcb0 = f["fc_b0"][None, :].astype(NPDT)
    fcw1 = np.ascontiguousarray(
        f["fc_w1"].T.reshape(2, 128, 2).transpose(1, 0, 2)).astype(NPDT)
    fcb1 = f["fc_b1"][None, :].astype(NPDT)

    shared = dict(wrec0=wrec0, wx1=wx1, wrec1=wrec1, w0aug=w0aug, bias1=bias1,
                  fcw0=fcw0, fcb0=fcb0, fcw1=fcw1, fcb1=fcb1)
    in_maps = []
    for c in range(N_CORES):
        u = f["u_seq"][c * BC:(c + 1) * BC, :t_steps, :]  # [BC, t, IN]
        uaug = np.concatenate(
            [u.transpose(2, 1, 0),
             np.ones((1, t_steps, BC), np.float32)], axis=0)
        # One zero pad step: superstep t_steps-1 opens (never-read) gates
        # for step t_steps.
        uaug = np.concatenate(
            [uaug, np.zeros((IN + 1, 1, BC), np.float32)], axis=1)
        in_maps.append(dict(shared, ut=np.ascontiguousarray(uaug.astype(NPDT))))
    return in_maps


# Empirical per-instruction sync-wait encoding capacity (walrus codegen
# rejects instructions over these; Tile scheduling is not deterministic
# across processes, so validate each build and reschedule on violation).
_WAIT_CAPS = {
    "InstTensorTensor": 1, "InstTensorCopy": 1, "InstStreamTranspose": 1,
    "InstTensorScalarPtr": 1, "InstTensorScalarAffineSelect": 1,
    "InstMatmult": 2, "InstLdweights": 2, "InstActivation": 2,
    "InstMemset": 2, "InstDMACopy": 2, "InstReciprocal": 1,
}


def _wait_violations(nc):
    bad = []
    for blk in nc.m.functions[0].blocks:
        for inst in blk.instructions:
            cap = _WAIT_CAPS.get(type(inst).__name__)
            if cap is None:
                continue
            w = inst.concise().count("wait:")
            if w > cap:
                bad.append((inst.name, type(inst).__name__, w))
    return bad


def run(inputs: dict, t_steps: int = T, trace: bool = False, reps: int = 1):
    nc = build_program(t_steps, reps)
    in_maps = prep_inputs(inputs, t_steps)
    r = run_bass_kernel_spmd(nc, in_maps, core_ids=list(range(N_CORES)),
                             trace=trace)
    out = np.concatenate([r.results[c]["out"] for c in range(N_CORES)], axis=0)
    return out.astype(np.float32), r


def kernel(**inputs) -> np.ndarray:
    out, _ = run(inputs, T)
    return out


# revision 36
# speedup vs baseline: 1.0304x; 1.0304x over previous
"""Trainium2 Bass kernel for a 2-layer LSTM (B=256, T=512, IN=8, H=512) + FC head.

Strategy: data-parallel over batch (32 per core x 8 cores). Per core, one
software-pipelined loop over supersteps s: layer-0 computes step s while
layer-1 computes step s-1 (so layer-1's matmuls never wait on this step's
elementwise chain). Gates are computed with the batch (32) as the PE
stationary free dim, one gate per PE column-group, so the four gates land
partition-stacked [i|f|o|g] x 32batch in PSUM and biases/input-projections
are folded in as extra accumulating matmuls (ones-row trick). The scalar
engine applies sigmoid/tanh straight out of PSUM, the vector engine does the
c/h updates, and h is transposed back to [hidden, batch] stationary layout
with PE identity matmuls.

The superstep loop is a hardware For_i loop (SPB supersteps per body) so the
program size is O(1) in T instead of O(T): the fully-unrolled version's NEFF
was ~30k instructions and its load/transfer time dominated wall clock.
"""

import sys
from contextlib import ExitStack

import numpy as np

try:
    import concourse.bass as bass  # noqa: F401
except ImportError:
    sys.path.insert(0, "/opt/trn_rl_repo")

import ml_dtypes
import concourse.bacc as bacc
import concourse.bass as bass
import concourse.mybir as mybir
import concourse.tile as tile
from concourse.bass import ds
from concourse.bass_utils import run_bass_kernel_spmd
from concourse.masks import make_identity
from concourse.tile_rust import add_dep_helper

B, T, IN, H = 256, 512, 8, 512
N_CORES = 8
BC = B // N_CORES  # 32 batch rows per core
MAXV, MINV = 4.2, 2.5

AF = mybir.ActivationFunctionType
F32 = mybir.dt.float32

# Stream dtype for matmul operands (weights, h, u). bf16 = 1 PE cycle/row.
DT = mybir.dt.bfloat16
NPDT = ml_dtypes.bfloat16

# Gate blocks in torch order: i[0:H], f[H:2H], g[2H:3H], o[3H:4H].
# On-chip layout order is [i, f, o, g] so the three sigmoids are one
# contiguous 96-partition block. PERM maps layout order -> torch rows.
PERM = np.concatenate(
    [np.arange(0, H), np.arange(H, 2 * H), np.arange(3 * H, 4 * H),
     np.arange(2 * H, 3 * H)]
)


def _shuffle_kxn(w_t: np.ndarray, n: int) -> np.ndarray:
    """[512, n] (contraction-major) -> [128, 4, n] SBUF layout (chunk k = rows
    128k:128k+128 on partition p)."""
    return np.ascontiguousarray(w_t.reshape(4, 128, n).transpose(1, 0, 2))


def build_program(t_steps: int, reps: int = 1, virtual_steps: int = 0):
    """Emit the per-core Bass program. All 8 cores run this same program.
    Bacc (not plain Bass): its finalize() runs the compile passes that move
    matmul waits onto ldweights and split multi-wait instructions into
    event semaphores -- hardware instructions encode only one sync wait.

    virtual_steps > t_steps builds a TIMING variant: the identical program
    except the hardware loop runs (virtual_steps - spb)/spb iterations with
    the u_t staging copy always reading block 0 (data values recycle; the
    per-iteration instruction stream and timing are unchanged). Output is
    then numerically meaningless -- timing only."""
    nc = bacc.Bacc()

    # Supersteps per hardware-loop body. The loop covers s in [SPB, t_steps);
    # supersteps 0..SPB-1 are the peeled prologue (s=0 has no layer-1 work)
    # and s=t_steps the peeled epilogue (layer-1 only).
    spb = 8 if (t_steps % 8 == 0 and t_steps >= 16) else 1

    # --- DRAM parameters (declaration order = in_map key order is by name) ---
    dp = nc.declare_dram_parameter
    wrec0_d = dp("wrec0", [128, 4, 4 * H], DT, isOutput=False)
    wx1_d = dp("wx1", [128, 4, 4 * H], DT, isOutput=False)
    wrec1_d = dp("wrec1", [128, 4, 4 * H], DT, isOutput=False)
    w0aug_d = dp("w0aug", [IN + 1, 4 * H], DT, isOutput=False)
    bias1_d = dp("bias1", [1, 4 * H], DT, isOutput=False)
    ut_d = dp("ut", [IN + 1, t_steps + 1, BC], DT, isOutput=False)
    fcw0_d = dp("fcw0", [128, 4, 256], DT, isOutput=False)
    fcb0_d = dp("fcb0", [1, 256], DT, isOutput=False)
    fcw1_d = dp("fcw1", [128, 2, 2], DT, isOutput=False)
    fcb1_d = dp("fcb1", [1, 2], DT, isOutput=False)
    out_d = dp("out", [BC, 2], F32, isOutput=True)

    with tile.TileContext(nc) as tc, ExitStack() as ctx:
        const = ctx.enter_context(tc.tile_pool(name="const", bufs=1))
        st = ctx.enter_context(tc.tile_pool(name="state", bufs=8))
        work = ctx.enter_context(tc.tile_pool(name="work", bufs=4))
        pp = ctx.enter_context(tc.tile_pool(name="ps", bufs=3, space="PSUM"))
        pt = ctx.enter_context(tc.tile_pool(name="pst", bufs=1, space="PSUM"))
        pf = ctx.enter_context(tc.tile_pool(name="psf", bufs=1, space="PSUM"))
        up = ctx.enter_context(tc.tile_pool(name="ustage", bufs=1))

        # --- resident weights / inputs ---
        wrec0 = const.tile([128, 4, 4 * H], DT, tag="wrec0")
        wx1 = const.tile([128, 4, 4 * H], DT, tag="wx1")
        wrec1 = const.tile([128, 4, 4 * H], DT, tag="wrec1")
        w0aug = const.tile([IN + 1, 4 * H], DT, tag="w0aug")
        bias1 = const.tile([1, 4 * H], DT, tag="bias1")
        ut = const.tile([IN + 1, t_steps + 1, BC], DT, tag="ut")
        fcw0 = const.tile([128, 4, 256], DT, tag="fcw0")
        fcb0 = const.tile([1, 256], DT, tag="fcb0")
        fcw1 = const.tile([128, 2, 2], DT, tag="fcw1")
        fcb1 = const.tile([1, 2], DT, tag="fcb1")
        for sb, d in ((wrec0, wrec0_d), (wx1, wx1_d), (wrec1, wrec1_d),
                      (w0aug, w0aug_d), (bias1, bias1_d), (ut, ut_d),
                      (fcw0, fcw0_d), (fcb0, fcb0_d), (fcw1, fcw1_d),
                      (fcb1, fcb1_d)):
            nc.sync.dma_start(sb[:], d[:])

        # Preheat: 1-element matmuls touching every DMA-loaded tensor, so PE
        # observes each DMA-HW queue semaphore up front. Otherwise the first
        # PE consumer of a tensor carries an extra DMA wait on top of its
        # ACT/PE waits, and a Matmult encodes at most two sync waits.
        pre = pf.tile([1, 1], F32, tag="pfence")
        preheats = []
        for ap in (wrec0[0:1, 0, 0:1], wx1[0:1, 0, 0:1], wrec1[0:1, 0, 0:1],
                   w0aug[0:1, 0:1], bias1[0:1, 0:1], ut[0:1, 0, 0:1],
                   fcw0[0:1, 0, 0:1], fcb0[0:1, 0:1], fcw1[0:1, 0, 0:1],
                   fcb1[0:1, 0:1]):
            preheats.append(
                nc.tensor.matmul(pre[:], ap, ap, start=True, stop=True))

        ident = const.tile([32, 32], DT, tag="ident")
        make_identity(nc, ident[:])
        ones = const.tile([1, BC], DT, tag="ones")
        nc.vector.memset(ones[:], 1.0)
        out_bias = const.tile([BC, 1], F32, tag="out_bias")
        nc.vector.memset(out_bias[:], (MAXV - MINV) / 2 + MINV)

        early_mms = []

        def gps(ps, g):
            return ps[32 * g:32 * g + 32, :]

        # Mutable pipeline state shared by superstep emissions.
        sv = {}

        def start_l0(ut_ap, collect=None):
            """Open next step's layer-0 gate accumulation: [u_t, 1] @ w0aug
            (biases ride the ones row). Independent of any recent h, so the
            PE chews it while ACT/DVE run the current elementwise chains.
            The sync dep on the previous superstep's sigmoid throttles the
            scheduler: these matmuls have no data deps, and unthrottled it
            hoists several supersteps' opens, blowing the PSUM ring."""
            ps0 = pp.tile([128, H], F32, tag="ps0", name="ps0")
            for g in range(4):
                r = nc.tensor.matmul(gps(ps0, g), ut_ap,
                                     w0aug[:, 512 * g:512 * g + 512],
                                     start=True, stop=False,
                                     tile_position=(0, 32 * g))
                if g == 0 and sv.get("sig0_prev") is not None:
                    add_dep_helper(r.ins, sv["sig0_prev"].ins, sync=True,
                                   reason="throttle-ps0-open")
                if collect is not None:
                    collect.append(r)
            sv["ps0n"] = ps0

        def start_l1(collect=None):
            """Open next l1 step's gate accumulation: bias1 + h0 @ wx1.
            h0 here is one superstep old -- ready long before this runs."""
            ps1 = pp.tile([128, H], F32, tag="ps1", name="ps1")
            for g in range(4):
                r = nc.tensor.matmul(gps(ps1, g), ones[:],
                                     bias1[:, 512 * g:512 * g + 512],
                                     start=True, stop=False,
                                     tile_position=(0, 32 * g))
                if g == 0 and sv.get("sig1_prev") is not None:
                    add_dep_helper(r.ins, sv["sig1_prev"].ins, sync=True,
                                   reason="throttle-ps1-open")
                if collect is not None:
                    collect.append(r)
            for k in range(4):
                for g in range(4):
                    r = nc.tensor.matmul(gps(ps1, g),
                                         sv["h0T"][:, 32 * k:32 * k + 32],
                                         wx1[:, k, 512 * g:512 * g + 512],
                                         start=False, stop=False,
                                         tile_position=(0, 32 * g))
                    if collect is not None:
                        collect.append(r)
            sv["ps1n"] = ps1

        def emit_superstep(ut_next_ap, l0, l1, l0n, l1n, collect=None):
            """One steady-state superstep s, layers phase-shifted so each
            layer's elementwise chain overlaps the other layer's matmuls:

              PE : wrec0(s) | wrec1(s-2) | start l0(s+1), l1(s-1) | transposes
              ACT:      tanh/sig l0 | tanh/sig l1 | tanh-c l0 | tanh-c l1
              DVE:          c/h updates l0 | c/h updates l1 | h*T copies

            Matmuls go round-robin across the four gates: each gate owns one
            32-wide PE column-group (tile_position) and col-groups stream
            CONCURRENTLY through separate XBUSes when adjacent in program
            order (measured ~2.4-3x for 3-4 way col tiling)."""

            def gmm(*args, **kw):
                r = nc.tensor.matmul(*args, **kw)
                if collect is not None:
                    collect.append(r)
                return r

            ps0, ps1 = sv.get("ps0n"), sv.get("ps1n")
            # --- PE: close layer-0 gates for step s: + h0(s-1) @ wrec0 ---
            if l0:
                for k in range(4):
                    for g in range(4):
                        gmm(gps(ps0, g), sv["h0T"][:, 32 * k:32 * k + 32],
                            wrec0[:, k, 512 * g:512 * g + 512],
                            start=False, stop=(k == 3),
                            tile_position=(0, 32 * g))
            # --- ACT: layer-0 nonlinearities straight out of PSUM. Sigmoid
            # first: f*c_prev needs only the sigmoid, so it runs on DVE
            # while ACT still computes tanh(g). ---
            gt = work.tile([BC, 2 * H], DT, tag="gt")
            sig = work.tile([96, 2 * H], DT, tag="sig")  # [i|f|o] x batch
            if l0:
                nc.scalar.activation(gt[:, 0:H], ps0[96:128, :], AF.Tanh)
                sv["sig0_prev"] = nc.scalar.activation(
                    sig[:, 0:H], ps0[0:96, :], AF.Sigmoid)
            # --- PE: close layer-1 gates for step s-2: + h1(s-3) @ wrec1 ---
            if l1:
                for k in range(4):
                    for g in range(4):
                        gmm(gps(ps1, g), sv["h1T"][:, 32 * k:32 * k + 32],
                            wrec1[:, k, 512 * g:512 * g + 512],
                            start=False, stop=(k == 3),
                            tile_position=(0, 32 * g))
            # --- DVE: layer-0 c update ---
            ig = work.tile([BC, 2 * H], DT, tag="ig")
            fcp = work.tile([BC, 2 * H], F32, tag="fcp")
            c_new = st.tile([64, 2 * H], F32, tag="c", name="c")
            if l0:
                nc.vector.tensor_mul(ig[:, 0:H], sig[0:32, 0:H], gt[:, 0:H])
                nc.vector.tensor_mul(fcp[:, 0:H], sig[32:64, 0:H],
                                     sv["c_prev"][32:64, 0:H])
                nc.vector.tensor_add(c_new[32:64, 0:H], fcp[:, 0:H],
                                     ig[:, 0:H])
            # --- ACT: layer-1 nonlinearities ---
            if l1:
                nc.scalar.activation(gt[:, H:2 * H], ps1[96:128, :], AF.Tanh)
                sv["sig1_prev"] = nc.scalar.activation(
                    sig[:, H:2 * H], ps1[0:96, :], AF.Sigmoid)
            elif l0:
                # Layer-1 idle (pipeline fill): carry its c half forward as
                # zero so the first l1 superstep reads c=0.
                nc.vector.memset(c_new[32:64, H:2 * H], 0.0)
            # --- PE: open next supersteps' gate accumulations ---
            if l0n:
                start_l0(ut_next_ap, collect)
            if l1n:
                start_l1(collect)
            # --- DVE: layer-1 c update ---
            if l1:
                nc.vector.tensor_mul(ig[:, H:2 * H], sig[0:32, H:2 * H],
                                     gt[:, H:2 * H])
                nc.vector.tensor_mul(fcp[:, H:2 * H], sig[32:64, H:2 * H],
                                     sv["c_prev"][32:64, H:2 * H])
            # --- ACT tanh(c) + DVE h for layer 0, then layer 1 ---
            th = work.tile([96, 2 * H], DT, tag="th")
            hn = work.tile([BC, 2 * H], DT, tag="hn")
            hps = pt.tile([128, 256], DT, tag="hps")
            if l0:
                nc.scalar.activation(th[64:96, 0:H], c_new[32:64, 0:H],
                                     AF.Tanh)
            if l1:
                nc.vector.tensor_add(c_new[32:64, H:2 * H], fcp[:, H:2 * H],
                                     ig[:, H:2 * H])
            if l0:
                nc.vector.tensor_mul(hn[:, 0:H], sig[64:96, 0:H],
                                     th[64:96, 0:H])
                # PE transpose back to [hid, batch] stationary layout; DVE
                # (not ACT: 0.11us vs 0.4us) copies PSUM -> SBUF.
                for k in range(4):
                    nc.tensor.transpose(hps[:, 32 * k:32 * k + 32],
                                        hn[:, 128 * k:128 * k + 128], ident[:])
                sv["h0T"] = st.tile([128, 128], DT, tag="h0T", name="h0T")
                nc.vector.tensor_copy(sv["h0T"][:], hps[:, 0:128])
            if l1:
                nc.scalar.activation(th[64:96, H:2 * H], c_new[32:64, H:2 * H],
                                     AF.Tanh)
                nc.vector.tensor_mul(hn[:, H:2 * H], sig[64:96, H:2 * H],
                                     th[64:96, H:2 * H])
                for k in range(4):
                    nc.tensor.transpose(hps[:, 128 + 32 * k:160 + 32 * k],
                                        hn[:, H + 128 * k:H + 128 * k + 128],
                                        ident[:])
                sv["h1T"] = st.tile([128, 128], DT, tag="h1T", name="h1T")
                nc.vector.tensor_copy(sv["h1T"][:], hps[:, 128:256])
            sv["c_prev"] = c_new

            # PE-clock fence: a 1x1 matmul (own PSUM bank) + DVE copy makes
            # DVE observe a PE tick that transitively follows this step's
            # transposes, so later WAR-vs-PE waits on reused work slots are
            # pre-covered and TensorTensors keep to their one sync wait.
            lo = 0 if l0 else H
            pfence = pf.tile([1, 1], F32, tag="pfence")
            nc.tensor.matmul(pfence[:], hn[0:1, lo:lo + 1],
                             ones[0:1, 0:1], start=True, stop=True)
            fence = work.tile([1, 1], F32, tag="fence")
            nc.vector.tensor_copy(fence[:], pfence[:])

        for rep in range(reps):
          # --- initial state (h/c zero) ---
          # c lives on partitions 32:64 so that f*c pairs sigmoid(f) (block
          # 32:64 of sig_t) with an equal base partition -- walrus requires
          # SBUF x SBUF tensor_tensor inputs to share their start partition.
          sv["h0T"] = st.tile([128, 128], DT, tag="h0T", name="h0T")
          sv["h1T"] = st.tile([128, 128], DT, tag="h1T", name="h1T")
          sv["c_prev"] = st.tile([64, 2 * H], F32, tag="c", name="c")
          nc.vector.memset(sv["h0T"][:], 0.0)
          nc.vector.memset(sv["h1T"][:], 0.0)
          nc.vector.memset(sv["c_prev"][32:64, :], 0.0)

          # Superstep s runs: close l0 gates for step s, close l1 gates for
          # step s-2, and open the next supersteps' gate accumulations
          # (l0 step s+1, l1 step s-1 -- whose wx1 consumes the h0 produced
          # one superstep ago, so it never waits on a fresh h).
          # Prime l0 step 0, then prologue supersteps 0..spb-1 (unrolled).
          start_l0(ut[:, 0, :], collect=early_mms)
          for s in range(spb):
              emit_superstep(ut[:, s + 1, :], True, s >= 2, True, s >= 1,
                             collect=early_mms if s <= 1 else None)

          # Steady state: hardware loop over supersteps [spb, t_steps).
          # walrus does not support register offsets in ldweights, so the
          # body's u_t stationaries are staged: one DVE copy per body pulls
          # ut[:, iv:iv+spb+1, :] into a fixed tile the matmuls slice
          # statically (spb+1 wide: superstep iv+k opens step iv+k+1; ut is
          # padded by one zero step so the final open reads in-bounds).
          # hint_engines=(PE,): the body is >256 PE instructions (more than
          # one IRAM block), so arm the back-edge prefetcher.
          loop_end = max(virtual_steps, t_steps)
          if loop_end > spb:
              with tc.For_i(spb, loop_end, spb,
                            hint_engines=(mybir.EngineType.PE,)) as iv:
                  ust = up.tile([IN + 1, (spb + 1) * BC], DT, tag="ust",
                                name="ust")
                  if virtual_steps > t_steps:
                      nc.vector.tensor_copy(ust[:], ut[:, 0:spb + 1, :])
                  else:
                      nc.vector.tensor_copy(ust[:], ut[:, ds(iv, spb + 1), :])
                  for k in range(spb):
                      emit_superstep(ust[:, BC * (k + 1):BC * (k + 2)],
                                     True, True, True, True)

          # Epilogue supersteps t_steps, t_steps+1: layer-1 only (drains
          # the lag-2 pipeline: l1 steps t_steps-2 and t_steps-1).
          emit_superstep(None, False, True, False, True)
          emit_superstep(None, False, True, False, False)

        # Preheat matmuls must precede the early-step matmuls in PE program
        # order so the DMA-queue semaphore waits land on the preheats.
        for p in preheats:
            for m_ in early_mms:
                add_dep_helper(p.ins, m_.ins, sync=True, reason="preheat-first")

        # --- FC head on final h1 (h1T holds it in [hid, batch] layout) ---
        psf = pp.tile([128, H], F32, tag="ps0", name="psf")
        nc.tensor.matmul(psf[0:BC, 0:256], ones[:], fcb0[:],
                         start=True, stop=False)
        for k in range(4):
            nc.tensor.matmul(psf[0:BC, 0:256], sv["h1T"][:, 32 * k:32 * k + 32],
                             fcw0[:, k, :], start=False, stop=(k == 3))
        z = work.tile([BC, 256], DT, tag="z")
        nc.scalar.activation(z[:], psf[0:BC, 0:256], AF.Tanh)
        zps = pt.tile([128, 256], DT, tag="hps", name="zps")
        for k in range(2):
            nc.tensor.transpose(zps[:, 32 * k:32 * k + 32],
                                z[:, 128 * k:128 * k + 128], ident[:])
        zt = work.tile([128, 64], DT, tag="zt")
        nc.scalar.copy(zt[:], zps[:, 0:64])
        psg = pp.tile([128, H], F32, tag="ps1", name="psg")
        nc.tensor.matmul(psg[0:BC, 0:2], ones[:], fcb1[:],
                         start=True, stop=False)
        for k in range(2):
            nc.tensor.matmul(psg[0:BC, 0:2], zt[:, 32 * k:32 * k + 32],
                             fcw1[:, k, :], start=False, stop=(k == 1))
        res = work.tile([BC, 2], F32, tag="res")
        # out = (x + 1)/2 * (MAXV-MINV) + MINV = 0.85*x + 3.35
        nc.scalar.activation(res[:], psg[0:BC, 0:2], AF.Identity,
                             bias=out_bias[:], scale=(MAXV - MINV) / 2)
        nc.sync.dma_start(out_d[:], res[:])

    nc.finalize()
    return nc


def prep_inputs(inputs: dict, t_steps: int):
    """Host-side: transpose/permute/convert everything into device layouts.
    Returns the per-core in_maps list."""
    f = {k: np.asarray(v, np.float32) for k, v in inputs.items()}
    wrec0 = _shuffle_kxn(f["w_hh_0"].T[:, PERM], 4 * H).astype(NPDT)
    wx1 = _shuffle_kxn(f["w_ih_1"].T[:, PERM], 4 * H).astype(NPDT)
    wrec1 = _shuffle_kxn(f["w_hh_1"].T[:, PERM], 4 * H).astype(NPDT)
    w0aug = np.concatenate(
        [f["w_ih_0"].T, (f["b_ih_0"] + f["b_hh_0"])[None, :]], axis=0
    )[:, PERM].astype(NPDT)
    bias1 = (f["b_ih_1"] + f["b_hh_1"])[PERM][None, :].astype(NPDT)
    fcw0 = _shuffle_kxn(f["fc_w0"].T, 256).astype(NPDT)
    fcb0 = f["fc_b0"][None, :].astype(NPDT)
    fcw1 = np.ascontiguousarray(
        f["fc_w1"].T.reshape(2, 128, 2).transpose(1, 0, 2)).astype(NPDT)
    fcb1 = f["fc_b1"][None, :].astype(NPDT)

    shared = dict(wrec0=wrec0, wx1=wx1, wrec1=wrec1, w0aug=w0aug, bias1=bias1,
                  fcw0=fcw0, fcb0=fcb0, fcw1=fcw1, fcb1=fcb1)
    in_maps = []
    for c in range(N_CORES):
        u = f["u_seq"][c * BC:(c + 1) * BC, :t_steps, :]  # [BC, t, IN]
        uaug = np.concatenate(
            [u.transpose(2, 1, 0),
             np.ones((1, t_steps, BC), np.float32)], axis=0)
        # One zero pad step: superstep t_steps-1 opens (never-read) gates
        # for step t_steps.
        uaug = np.concatenate(
            [uaug, np.zeros((IN + 1, 1, BC), np.float32)], axis=1)
        in_maps.append(dict(shared, ut=np.ascontiguousarray(uaug.astype(NPDT))))
    return in_maps


# Empirical per-instruction sync-wait encoding capacity (walrus codegen
# rejects instructions over these; Tile scheduling is not deterministic
# across processes, so validate each build and reschedule on violation).
_WAIT_CAPS = {
    "InstTensorTensor": 1, "InstTensorCopy": 1, "InstStreamTranspose": 1,
    "InstTensorScalarPtr": 1, "InstTensorScalarAffineSelect": 1,
    "InstMatmult": 2, "InstLdweights": 2, "InstActivation": 2,
    "InstMemset": 2, "InstDMACopy": 2, "InstReciprocal": 1,
}


def _wait_violations(nc):
    bad = []
    for blk in nc.m.functions[0].blocks:
        for inst in blk.instructions:
            cap = _WAIT_CAPS.get(type(inst).__name__)
            if cap is None:
                continue
            w = inst.concise().count("wait:")
            if w > cap:
                bad.append((inst.name, type(inst).__name__, w))
    return bad


def run(inputs: dict, t_steps: int = T, trace: bool = False, reps: int = 1):
    nc = build_program(t_steps, reps)
    in_maps = prep_inputs(inputs, t_steps)
    r = run_bass_kernel_spmd(nc, in_maps, core_ids=list(range(N_CORES)),
                             trace=trace)
    out = np.concatenate([r.results[c]["out"] for c in range(N_CORES)], axis=0)
    return out.astype(np.float32), r


def kernel(**inputs) -> np.ndarray:
    out, _ = run(inputs, T)
    return out


# revision 40
# speedup vs baseline: 1.0374x; 1.0069x over previous
"""Trainium2 Bass kernel for a 2-layer LSTM (B=256, T=512, IN=8, H=512) + FC head.

Strategy: data-parallel over batch (32 per core x 8 cores). Per core, one
software-pipelined loop over supersteps s: layer-0 computes step s while
layer-1 computes step s-1 (so layer-1's matmuls never wait on this step's
elementwise chain). Gates are computed with the batch (32) as the PE
stationary free dim, one gate per PE column-group, so the four gates land
partition-stacked [i|f|o|g] x 32batch in PSUM and biases/input-projections
are folded in as extra accumulating matmuls (ones-row trick). The scalar
engine applies sigmoid/tanh straight out of PSUM, the vector engine does the
c/h updates, and h is transposed back to [hidden, batch] stationary layout
with PE identity matmuls.

The superstep loop is a hardware For_i loop (SPB supersteps per body) so the
program size is O(1) in T instead of O(T): the fully-unrolled version's NEFF
was ~30k instructions and its load/transfer time dominated wall clock.
"""

import sys
from contextlib import ExitStack

import numpy as np

try:
    import concourse.bass as bass  # noqa: F401
except ImportError:
    sys.path.insert(0, "/opt/trn_rl_repo")

import ml_dtypes
import concourse.bacc as bacc
import concourse.bass as bass
import concourse.mybir as mybir
import concourse.tile as tile
from concourse.bass import ds
from concourse.bass_utils import run_bass_kernel_spmd
from concourse.masks import make_identity
from concourse.tile_rust import add_dep_helper

B, T, IN, H = 256, 512, 8, 512

# Diagnostics for virtual_steps timing builds (output WRONG when used):
# RELAX=True reads stale tiles in recurrence matmuls/transposes (measures
# the stall-free throughput ceiling); MMN<512 narrows every gate matmul's
# streamed columns (same instruction count, less data -- separates PE
# dispatch cost from stream cost).
RELAX = False
MMN = 512
N_CORES = 8
BC = B // N_CORES  # 32 batch rows per core
MAXV, MINV = 4.2, 2.5

AF = mybir.ActivationFunctionType
F32 = mybir.dt.float32

# Stream dtype for matmul operands (weights, h, u). bf16 = 1 PE cycle/row.
DT = mybir.dt.bfloat16
NPDT = ml_dtypes.bfloat16

# Gate blocks in torch order: i[0:H], f[H:2H], g[2H:3H], o[3H:4H].
# On-chip layout order is [i, f, o, g] so the three sigmoids are one
# contiguous 96-partition block. PERM maps layout order -> torch rows.
PERM = np.concatenate(
    [np.arange(0, H), np.arange(H, 2 * H), np.arange(3 * H, 4 * H),
     np.arange(2 * H, 3 * H)]
)


def _shuffle_kxn(w_t: np.ndarray, n: int) -> np.ndarray:
    """[512, n] (contraction-major) -> [128, 4, n] SBUF layout (chunk k = rows
    128k:128k+128 on partition p)."""
    return np.ascontiguousarray(w_t.reshape(4, 128, n).transpose(1, 0, 2))


def build_program(t_steps: int, reps: int = 1, virtual_steps: int = 0):
    """Emit the per-core Bass program. All 8 cores run this same program.
    Bacc (not plain Bass): its finalize() runs the compile passes that move
    matmul waits onto ldweights and split multi-wait instructions into
    event semaphores -- hardware instructions encode only one sync wait.

    virtual_steps > t_steps builds a TIMING variant: the identical program
    except the hardware loop runs (virtual_steps - spb)/spb iterations with
    the u_t staging copy always reading block 0 (data values recycle; the
    per-iteration instruction stream and timing are unchanged). Output is
    then numerically meaningless -- timing only."""
    nc = bacc.Bacc()

    # Supersteps per hardware-loop body. The loop covers s in [SPB, t_steps);
    # supersteps 0..SPB-1 are the peeled prologue (s=0 has no layer-1 work)
    # and s=t_steps the peeled epilogue (layer-1 only).
    spb = 8 if (t_steps % 8 == 0 and t_steps >= 16) else 1

    # --- DRAM parameters (declaration order = in_map key order is by name) ---
    dp = nc.declare_dram_parameter
    wrec0_d = dp("wrec0", [128, 4, 4 * H], DT, isOutput=False)
    wx1_d = dp("wx1", [128, 4, 4 * H], DT, isOutput=False)
    wrec1_d = dp("wrec1", [128, 4, 4 * H], DT, isOutput=False)
    w0aug_d = dp("w0aug", [IN + 1, 4 * H], DT, isOutput=False)
    bias1_d = dp("bias1", [1, 4 * H], DT, isOutput=False)
    ut_d = dp("ut", [IN + 1, t_steps + 1, BC], DT, isOutput=False)
    fcw0_d = dp("fcw0", [128, 4, 256], DT, isOutput=False)
    fcb0_d = dp("fcb0", [1, 256], DT, isOutput=False)
    fcw1_d = dp("fcw1", [128, 2, 2], DT, isOutput=False)
    fcb1_d = dp("fcb1", [1, 2], DT, isOutput=False)
    out_d = dp("out", [BC, 2], F32, isOutput=True)

    with tile.TileContext(nc) as tc, ExitStack() as ctx:
        const = ctx.enter_context(tc.tile_pool(name="const", bufs=1))
        st = ctx.enter_context(tc.tile_pool(name="state", bufs=8))
        work = ctx.enter_context(tc.tile_pool(name="work", bufs=4))
        pp = ctx.enter_context(tc.tile_pool(name="ps", bufs=3, space="PSUM"))
        pt = ctx.enter_context(tc.tile_pool(name="pst", bufs=1, space="PSUM"))
        pf = ctx.enter_context(tc.tile_pool(name="psf", bufs=1, space="PSUM"))
        up = ctx.enter_context(tc.tile_pool(name="ustage", bufs=1))

        # --- resident weights / inputs ---
        wrec0 = const.tile([128, 4, 4 * H], DT, tag="wrec0")
        wx1 = const.tile([128, 4, 4 * H], DT, tag="wx1")
        wrec1 = const.tile([128, 4, 4 * H], DT, tag="wrec1")
        w0aug = const.tile([IN + 1, 4 * H], DT, tag="w0aug")
        bias1 = const.tile([1, 4 * H], DT, tag="bias1")
        ut = const.tile([IN + 1, t_steps + 1, BC], DT, tag="ut")
        fcw0 = const.tile([128, 4, 256], DT, tag="fcw0")
        fcb0 = const.tile([1, 256], DT, tag="fcb0")
        fcw1 = const.tile([128, 2, 2], DT, tag="fcw1")
        fcb1 = const.tile([1, 2], DT, tag="fcb1")
        for sb, d in ((wrec0, wrec0_d), (wx1, wx1_d), (wrec1, wrec1_d),
                      (w0aug, w0aug_d), (bias1, bias1_d), (ut, ut_d),
                      (fcw0, fcw0_d), (fcb0, fcb0_d), (fcw1, fcw1_d),
                      (fcb1, fcb1_d)):
            nc.sync.dma_start(sb[:], d[:])

        # Preheat: 1-element matmuls touching every DMA-loaded tensor, so PE
        # observes each DMA-HW queue semaphore up front. Otherwise the first
        # PE consumer of a tensor carries an extra DMA wait on top of its
        # ACT/PE waits, and a Matmult encodes at most two sync waits.
        pre = pf.tile([1, 1], F32, tag="pfence")
        preheats = []
        for ap in (wrec0[0:1, 0, 0:1], wx1[0:1, 0, 0:1], wrec1[0:1, 0, 0:1],
                   w0aug[0:1, 0:1], bias1[0:1, 0:1], ut[0:1, 0, 0:1],
                   fcw0[0:1, 0, 0:1], fcb0[0:1, 0:1], fcw1[0:1, 0, 0:1],
                   fcb1[0:1, 0:1]):
            preheats.append(
                nc.tensor.matmul(pre[:], ap, ap, start=True, stop=True))

        ident = const.tile([32, 32], DT, tag="ident")
        make_identity(nc, ident[:])
        ones = const.tile([1, BC], DT, tag="ones")
        nc.vector.memset(ones[:], 1.0)
        out_bias = const.tile([BC, 1], F32, tag="out_bias")
        nc.vector.memset(out_bias[:], (MAXV - MINV) / 2 + MINV)

        early_mms = []

        def gps(ps, g):
            return ps[32 * g:32 * g + 32, :]

        # Mutable pipeline state shared by superstep emissions.
        sv = {}

        def start_l0(ut_ap, collect=None):
            """Open next step's layer-0 gate accumulation: [u_t, 1] @ w0aug
            (biases ride the ones row). Independent of any recent h, so the
            PE chews it while ACT/DVE run the current elementwise chains.
            The sync dep on the previous superstep's sigmoid throttles the
            scheduler: these matmuls have no data deps, and unthrottled it
            hoists several supersteps' opens, blowing the PSUM ring."""
            ps0 = pp.tile([128, H], F32, tag="ps0", name="ps0")
            for g in range(4):
                r = nc.tensor.matmul(gps(ps0, g)[:, 0:MMN], ut_ap,
                                     w0aug[:, 512 * g:512 * g + MMN],
                                     start=True, stop=False,
                                     tile_position=(0, 32 * g))
                if g == 0 and sv.get("sig0_prev") is not None:
                    add_dep_helper(r.ins, sv["sig0_prev"].ins, sync=True,
                                   reason="throttle-ps0-open")
                if collect is not None:
                    collect.append(r)
            sv["ps0n"] = ps0

        def start_l1(collect=None):
            """Open next l1 step's gate accumulation: bias1 + h0 @ wx1.
            h0 here is one superstep old -- ready long before this runs."""
            ps1 = pp.tile([128, H], F32, tag="ps1", name="ps1")
            for g in range(4):
                r = nc.tensor.matmul(gps(ps1, g)[:, 0:MMN], ones[:],
                                     bias1[:, 512 * g:512 * g + MMN],
                                     start=True, stop=False,
                                     tile_position=(0, 32 * g))
                if g == 0 and sv.get("sig1_prev") is not None:
                    add_dep_helper(r.ins, sv["sig1_prev"].ins, sync=True,
                                   reason="throttle-ps1-open")
                if collect is not None:
                    collect.append(r)
            for k in range(4):
                for g in range(4):
                    r = nc.tensor.matmul(gps(ps1, g)[:, 0:MMN],
                                         sv["h0T"][:, 32 * k:32 * k + 32],
                                         wx1[:, k, 512 * g:512 * g + MMN],
                                         start=False, stop=False,
                                         tile_position=(0, 32 * g))
                    if collect is not None:
                        collect.append(r)
            sv["ps1n"] = ps1

        def emit_superstep(ut_next_ap, l0, l1, l0n, l1n, collect=None):
            """One steady-state superstep s, layers phase-shifted so each
            layer's elementwise chain overlaps the other layer's matmuls:

              PE : wrec0(s) | wrec1(s-2) | start l0(s+1), l1(s-1) | transposes
              ACT:      tanh/sig l0 | tanh/sig l1 | tanh-c l0 | tanh-c l1
              DVE:          c/h updates l0 | c/h updates l1 | h*T copies

            Matmuls go round-robin across the four gates: each gate owns one
            32-wide PE column-group (tile_position) and col-groups stream
            CONCURRENTLY through separate XBUSes when adjacent in program
            order (measured ~2.4-3x for 3-4 way col tiling)."""

            def gmm(*args, **kw):
                r = nc.tensor.matmul(*args, **kw)
                if collect is not None:
                    collect.append(r)
                return r

            ps0, ps1 = sv.get("ps0n"), sv.get("ps1n")
            # --- PE: close layer-0 gates for step s: + h0(s-1) @ wrec0 ---
            if l0:
                h0src = sv["h0T_old"] if RELAX else sv["h0T"]
                for k in range(4):
                    for g in range(4):
                        gmm(gps(ps0, g)[:, 0:MMN],
                            h0src[:, 32 * k:32 * k + 32],
                            wrec0[:, k, 512 * g:512 * g + MMN],
                            start=False, stop=(k == 3),
                            tile_position=(0, 32 * g))
            # --- ACT: layer-0 nonlinearities straight out of PSUM. Sigmoid
            # first: f*c_prev needs only the sigmoid, so it runs on DVE
            # while ACT still computes tanh(g). ---
            gt = work.tile([BC, 2 * H], DT, tag="gt")
            sig = work.tile([96, 2 * H], DT, tag="sig")  # [i|f|o] x batch
            if l0:
                nc.scalar.activation(gt[:, 0:H], ps0[96:128, :], AF.Tanh)
                sv["sig0_prev"] = nc.scalar.activation(
                    sig[:, 0:H], ps0[0:96, :], AF.Sigmoid)
            # --- PE: close layer-1 gates for step s-2: + h1(s-3) @ wrec1 ---
            if l1:
                h1src = sv["h1T_old"] if RELAX else sv["h1T"]
                for k in range(4):
                    for g in range(4):
                        gmm(gps(ps1, g)[:, 0:MMN],
                            h1src[:, 32 * k:32 * k + 32],
                            wrec1[:, k, 512 * g:512 * g + MMN],
                            start=False, stop=(k == 3),
                            tile_position=(0, 32 * g))
            # --- DVE: layer-0 c update ---
            ig = work.tile([BC, 2 * H], DT, tag="ig")
            fcp = work.tile([BC, 2 * H], F32, tag="fcp")
            c_new = st.tile([64, 2 * H], F32, tag="c", name="c")
            if l0:
                nc.vector.tensor_mul(ig[:, 0:H], sig[0:32, 0:H], gt[:, 0:H])
                nc.vector.tensor_mul(fcp[:, 0:H], sig[32:64, 0:H],
                                     sv["c_prev"][32:64, 0:H])
                nc.vector.tensor_add(c_new[32:64, 0:H], fcp[:, 0:H],
                                     ig[:, 0:H])
            # --- ACT: layer-1 nonlinearities ---
            if l1:
                nc.scalar.activation(gt[:, H:2 * H], ps1[96:128, :], AF.Tanh)
                sv["sig1_prev"] = nc.scalar.activation(
                    sig[:, H:2 * H], ps1[0:96, :], AF.Sigmoid)
            elif l0:
                # Layer-1 idle (pipeline fill): carry its c half forward as
                # zero so the first l1 superstep reads c=0.
                nc.vector.memset(c_new[32:64, H:2 * H], 0.0)
            # --- PE: open next supersteps' gate accumulations ---
            if l0n:
                start_l0(ut_next_ap, collect)
            if l1n:
                start_l1(collect)
            # --- DVE: layer-1 c update ---
            if l1:
                nc.vector.tensor_mul(ig[:, H:2 * H], sig[0:32, H:2 * H],
                                     gt[:, H:2 * H])
                nc.vector.tensor_mul(fcp[:, H:2 * H], sig[32:64, H:2 * H],
                                     sv["c_prev"][32:64, H:2 * H])
            # --- ACT tanh(c) + DVE h for layer 0, then layer 1 ---
            th = work.tile([96, 2 * H], DT, tag="th")
            hn = work.tile([BC, 2 * H], DT, tag="hn")
            if l0:
                nc.scalar.activation(th[64:96, 0:H], c_new[32:64, 0:H],
                                     AF.Tanh)
            if l1:
                nc.vector.tensor_add(c_new[32:64, H:2 * H], fcp[:, H:2 * H],
                                     ig[:, H:2 * H])
            if l0:
                nc.vector.tensor_mul(hn[:, 0:H], sig[64:96, 0:H],
                                     th[64:96, 0:H])
                # DVE 32x32 block transposes put h straight back into the
                # [hid%128, (chunk, batch)] stationary layout -- no PE
                # transpose + PSUM bounce (saves ~1.5us/step of PE time and
                # two sync hops on the recurrence critical path).
                hnt = sv["hn_old"] if (RELAX and sv["hn_old"] is not None) else hn
                hr = hnt[:, 0:H].rearrange("p (k q j) -> p q k j", k=4, q=4,
                                           j=32)
                sv["h0T_old"] = sv["h0T"]
                sv["h0T"] = st.tile([128, 128], DT, tag="h0T", name="h0T")
                for q in range(4):
                    nc.vector.transpose(
                        sv["h0T"][32 * q:32 * q + 32, :].rearrange(
                            "p (k j) -> p k j", k=4, j=32), hr[:, q, :, :])
            if l1:
                nc.scalar.activation(th[64:96, H:2 * H], c_new[32:64, H:2 * H],
                                     AF.Tanh)
                nc.vector.tensor_mul(hn[:, H:2 * H], sig[64:96, H:2 * H],
                                     th[64:96, H:2 * H])
                hnt1 = sv["hn_old"] if (RELAX and sv["hn_old"] is not None) else hn
                hr1 = hnt1[:, H:2 * H].rearrange("p (k q j) -> p q k j", k=4,
                                                 q=4, j=32)
                sv["h1T_old"] = sv["h1T"]
                sv["h1T"] = st.tile([128, 128], DT, tag="h1T", name="h1T")
                for q in range(4):
                    nc.vector.transpose(
                        sv["h1T"][32 * q:32 * q + 32, :].rearrange(
                            "p (k j) -> p k j", k=4, j=32), hr1[:, q, :, :])
            sv["c_prev"] = c_new
            sv["hn_old"] = hn

            # PE-clock fence: a 1x1 matmul (own PSUM bank) + DVE copy makes
            # DVE observe a PE tick that transitively follows this step's
            # transposes, so later WAR-vs-PE waits on reused work slots are
            # pre-covered and TensorTensors keep to their one sync wait.
            lo = 0 if l0 else H
            pfence = pf.tile([1, 1], F32, tag="pfence")
            nc.tensor.matmul(pfence[:], hn[0:1, lo:lo + 1],
                             ones[0:1, 0:1], start=True, stop=True)
            fence = work.tile([1, 1], F32, tag="fence")
            nc.vector.tensor_copy(fence[:], pfence[:])

        for rep in range(reps):
          # --- initial state (h/c zero) ---
          # c lives on partitions 32:64 so that f*c pairs sigmoid(f) (block
          # 32:64 of sig_t) with an equal base partition -- walrus requires
          # SBUF x SBUF tensor_tensor inputs to share their start partition.
          sv["h0T"] = st.tile([128, 128], DT, tag="h0T", name="h0T")
          sv["h1T"] = st.tile([128, 128], DT, tag="h1T", name="h1T")
          sv["c_prev"] = st.tile([64, 2 * H], F32, tag="c", name="c")
          nc.vector.memset(sv["h0T"][:], 0.0)
          nc.vector.memset(sv["h1T"][:], 0.0)
          nc.vector.memset(sv["c_prev"][32:64, :], 0.0)
          sv["h0T_old"] = sv["h0T"]
          sv["h1T_old"] = sv["h1T"]
          sv["hn_old"] = None

          # Superstep s runs: close l0 gates for step s, close l1 gates for
          # step s-2, and open the next supersteps' gate accumulations
          # (l0 step s+1, l1 step s-1 -- whose wx1 consumes the h0 produced
          # one superstep ago, so it never waits on a fresh h).
          # Prime l0 step 0, then prologue supersteps 0..spb-1 (unrolled).
          start_l0(ut[:, 0, :], collect=early_mms)
          for s in range(spb):
              emit_superstep(ut[:, s + 1, :], True, s >= 2, True, s >= 1,
                             collect=early_mms if s <= 1 else None)

          # Steady state: hardware loop over supersteps [spb, t_steps).
          # walrus does not support register offsets in ldweights, so the
          # body's u_t stationaries are staged: one DVE copy per body pulls
          # ut[:, iv:iv+spb+1, :] into a fixed tile the matmuls slice
          # statically (spb+1 wide: superstep iv+k opens step iv+k+1; ut is
          # padded by one zero step so the final open reads in-bounds).
          # hint_engines=(PE,): the body is >256 PE instructions (more than
          # one IRAM block), so arm the back-edge prefetcher.
          loop_end = max(virtual_steps, t_steps)
          if loop_end > spb:
              with tc.For_i(spb, loop_end, spb,
                            hint_engines=(mybir.EngineType.PE,)) as iv:
                  ust = up.tile([IN + 1, (spb + 1) * BC], DT, tag="ust",
                                name="ust")
                  if virtual_steps > t_steps:
                      nc.vector.tensor_copy(ust[:], ut[:, 0:spb + 1, :])
                  else:
                      nc.vector.tensor_copy(ust[:], ut[:, ds(iv, spb + 1), :])
                  for k in range(spb):
                      emit_superstep(ust[:, BC * (k + 1):BC * (k + 2)],
                                     True, True, True, True)

          # Epilogue supersteps t_steps, t_steps+1: layer-1 only (drains
          # the lag-2 pipeline: l1 steps t_steps-2 and t_steps-1).
          emit_superstep(None, False, True, False, True)
          emit_superstep(None, False, True, False, False)

        # Preheat matmuls must precede the early-step matmuls in PE program
        # order so the DMA-queue semaphore waits land on the preheats.
        for p in preheats:
            for m_ in early_mms:
                add_dep_helper(p.ins, m_.ins, sync=True, reason="preheat-first")

        # --- FC head on final h1 (h1T holds it in [hid, batch] layout) ---
        psf = pp.tile([128, H], F32, tag="ps0", name="psf")
        nc.tensor.matmul(psf[0:BC, 0:256], ones[:], fcb0[:],
                         start=True, stop=False)
        for k in range(4):
            nc.tensor.matmul(psf[0:BC, 0:256], sv["h1T"][:, 32 * k:32 * k + 32],
                             fcw0[:, k, :], start=False, stop=(k == 3))
        z = work.tile([BC, 256], DT, tag="z")
        nc.scalar.activation(z[:], psf[0:BC, 0:256], AF.Tanh)
        zps = pt.tile([128, 256], DT, tag="hps", name="zps")
        for k in range(2):
            nc.tensor.transpose(zps[:, 32 * k:32 * k + 32],
                                z[:, 128 * k:128 * k + 128], ident[:])
        zt = work.tile([128, 64], DT, tag="zt")
        nc.scalar.copy(zt[:], zps[:, 0:64])
        psg = pp.tile([128, H], F32, tag="ps1", name="psg")
        nc.tensor.matmul(psg[0:BC, 0:2], ones[:], fcb1[:],
                         start=True, stop=False)
        for k in range(2):
            nc.tensor.matmul(psg[0:BC, 0:2], zt[:, 32 * k:32 * k + 32],
                             fcw1[:, k, :], start=False, stop=(k == 1))
        res = work.tile([BC, 2], F32, tag="res")
        # out = (x + 1)/2 * (MAXV-MINV) + MINV = 0.85*x + 3.35
        nc.scalar.activation(res[:], psg[0:BC, 0:2], AF.Identity,
                             bias=out_bias[:], scale=(MAXV - MINV) / 2)
        nc.sync.dma_start(out_d[:], res[:])

    nc.finalize()
    return nc


def prep_inputs(inputs: dict, t_steps: int):
    """Host-side: transpose/permute/convert everything into device layouts.
    Returns the per-core in_maps list."""
    f = {k: np.asarray(v, np.float32) for k, v in inputs.items()}
    wrec0 = _shuffle_kxn(f["w_hh_0"].T[:, PERM], 4 * H).astype(NPDT)
    wx1 = _shuffle_kxn(f["w_ih_1"].T[:, PERM], 4 * H).astype(NPDT)
    wrec1 = _shuffle_kxn(f["w_hh_1"].T[:, PERM], 4 * H).astype(NPDT)
    w0aug = np.concatenate(
        [f["w_ih_0"].T, (f["b_ih_0"] + f["b_hh_0"])[None, :]], axis=0
    )[:, PERM].astype(NPDT)
    bias1 = (f["b_ih_1"] + f["b_hh_1"])[PERM][None, :].astype(NPDT)
    fcw0 = _shuffle_kxn(f["fc_w0"].T, 256).astype(NPDT)
    fcb0 = f["fc_b0"][None, :].astype(NPDT)
    fcw1 = np.ascontiguousarray(
        f["fc_w1"].T.reshape(2, 128, 2).transpose(1, 0, 2)).astype(NPDT)
    fcb1 = f["fc_b1"][None, :].astype(NPDT)

    shared = dict(wrec0=wrec0, wx1=wx1, wrec1=wrec1, w0aug=w0aug, bias1=bias1,
                  fcw0=fcw0, fcb0=fcb0, fcw1=fcw1, fcb1=fcb1)
    in_maps = []
    for c in range(N_CORES):
        u = f["u_seq"][c * BC:(c + 1) * BC, :t_steps, :]  # [BC, t, IN]
        uaug = np.concatenate(
            [u.transpose(2, 1, 0),
             np.ones((1, t_steps, BC), np.float32)], axis=0)
        # One zero pad step: superstep t_steps-1 opens (never-read) gates
        # for step t_steps.
        uaug = np.concatenate(
            [uaug, np.zeros((IN + 1, 1, BC), np.float32)], axis=1)
        in_maps.append(dict(shared, ut=np.ascontiguousarray(uaug.astype(NPDT))))
    return in_maps


# Empirical per-instruction sync-wait encoding capacity (walrus codegen
# rejects instructions over these; Tile scheduling is not deterministic
# across processes, so validate each build and reschedule on violation).
_WAIT_CAPS = {
    "InstTensorTensor": 1, "InstTensorCopy": 1, "InstStreamTranspose": 1,
    "InstTensorScalarPtr": 1, "InstTensorScalarAffineSelect": 1,
    "InstMatmult": 2, "InstLdweights": 2, "InstActivation": 2,
    "InstMemset": 2, "InstDMACopy": 2, "InstReciprocal": 1,
}


def _wait_violations(nc):
    bad = []
    for blk in nc.m.functions[0].blocks:
        for inst in blk.instructions:
            cap = _WAIT_CAPS.get(type(inst).__name__)
            if cap is None:
                continue
            w = inst.concise().count("wait:")
            if w > cap:
                bad.append((inst.name, type(inst).__name__, w))
    return bad


def run(inputs: dict, t_steps: int = T, trace: bool = False, reps: int = 1):
    nc = build_program(t_steps, reps)
    in_maps = prep_inputs(inputs, t_steps)
    r = run_bass_kernel_spmd(nc, in_maps, core_ids=list(range(N_CORES)),
                             trace=trace)
    out = np.concatenate([r.results[c]["out"] for c in range(N_CORES)], axis=0)
    return out.astype(np.float32), r


def kernel(**inputs) -> np.ndarray:
    out, _ = run(inputs, T)
    return out
